# revision 2
# baseline (speedup 1.0000x reference)
"""Trainium2 Bass kernel for nn_Block_82111184765408 (pre-LN transformer block).

B=128, T=256, C=384, H=6, D=64, FF=1536. Data-parallel over batch across 8
NeuronCores (16 batches/core), batches processed in fused pairs (free dim 512).

v2: fp8(e4m3) DoubleRow matmuls for all C/FF contractions (weights scaled x32
on-device), bf16 transposes, bn_stats layernorm, causal-skip scores/exp,
additive -inf pre-masking on psum, parity-column softmax denominators with a
shared per-mo reciprocal broadcast, and double-buffered tiles so consecutive
batch pairs pipeline across engines.

Dataflow per pair: x (token-major f32) -> bn_stats LN1 -> htok bf16 ->
PE-transpose (bf16 identity) -> h1T fp8 [P,4,TP] (4th k-tile zero-padded,
gain/bias folded) -> QT/KT fp8 (DoubleRow) / V fp8 token-major (DoubleRow,
bias via ones-row matmul, parity ones cols 64/65 for denominators) ->
scoresT per head (causal-skipped, -1e30 premask on psum) -> exp fp8 ->
attn@V DoubleRow (denom rides in rows 64/65) -> per-mo reciprocal +
sel-matmul broadcast (x8 for fp8 range) -> AVT fp8 -> proj (DoubleRow) ->
transpose-back + residual f32 -> LN2 -> FFN (DoubleRow both layers, relu on
ACT) -> transpose + residual -> DMA out.
"""

import numpy as np

import concourse.bass as bass
import concourse.mybir as mybir
import concourse.tile as tile
from concourse import bacc
from concourse.bass_utils import run_bass_kernel_spmd
from concourse.masks import make_identity

P = 128
B, T, C, H, D = 128, 256, 384, 6, 64
FF = 4 * C
N_CORES = 8
B_LOCAL = B // N_CORES          # 16 batches per core
N_PAIRS = B_LOCAL // 2          # 8 pairs, free dim 512 per pair
TP = 2 * T                      # 512
CC = C // P                     # 3 feature chunks
FC = FF // P                    # 12 ffn chunks
EPS = 1e-5
SCALE = C ** -0.5
WS = 32.0                       # fp8 weight scale
AVS = 8.0                       # AVT scale (baked into sel2)

f32 = mybir.dt.float32
f32r = mybir.dt.float32r
bf16 = mybir.dt.bfloat16
f8 = mybir.dt.float8e4
AF = mybir.ActivationFunctionType
OP = mybir.AluOpType
PM = mybir.MatmulPerfMode


def build_nc(n_pairs=N_PAIRS, debug_outputs=False, repeat=1):
    nc = bacc.Bacc("TRN2", target_bir_lowering=False, debug=False)

    x_d = nc.declare_dram_parameter("x", [2 * n_pairs, T, C], f32, isOutput=False)
    ln1_g_d = nc.declare_dram_parameter("ln1_g", [C], f32, isOutput=False)
    ln1_b_d = nc.declare_dram_parameter("ln1_b", [C], f32, isOutput=False)
    Wk_d = nc.declare_dram_parameter("Wk", [H, C, D], f32, isOutput=False)
    bk_d = nc.declare_dram_parameter("bk", [H, D], f32, isOutput=False)
    Wq_d = nc.declare_dram_parameter("Wq", [H, C, D], f32, isOutput=False)
    bq_d = nc.declare_dram_parameter("bq", [H, D], f32, isOutput=False)
    Wv_d = nc.declare_dram_parameter("Wv", [H, C, D], f32, isOutput=False)
    bv_d = nc.declare_dram_parameter("bv", [H, D], f32, isOutput=False)
    Wp_d = nc.declare_dram_parameter("Wp", [C, C], f32, isOutput=False)
    bp_d = nc.declare_dram_parameter("bp", [C], f32, isOutput=False)
    ln2_g_d = nc.declare_dram_parameter("ln2_g", [C], f32, isOutput=False)
    ln2_b_d = nc.declare_dram_parameter("ln2_b", [C], f32, isOutput=False)
    W1_d = nc.declare_dram_parameter("W1", [C, FF], f32, isOutput=False)
    b1_d = nc.declare_dram_parameter("b1", [FF], f32, isOutput=False)
    W2_d = nc.declare_dram_parameter("W2", [FF, C], f32, isOutput=False)
    b2_d = nc.declare_dram_parameter("b2", [C], f32, isOutput=False)
    y_d = nc.declare_dram_parameter("y", [2 * n_pairs, T, C], f32, isOutput=True)
    dbg = {}
    if debug_outputs:
        for nm, shp in (("h1T0", [P, TP]), ("QT0", [P, TP]), ("KT0", [P, TP]),
                        ("V0", [P, TP]), ("E0", [P, TP]), ("AVT0", [P, TP]),
                        ("proj0", [P, TP]), ("out1", [P, 4 * C]),
                        ("h2T0", [P, TP]), ("FF0", [P, TP])):
            dbg[nm] = nc.declare_dram_parameter(nm, shp, f32, isOutput=True)

    with tile.TileContext(nc) as tc:
        with tc.tile_pool(name="const", bufs=1) as cst, \
             tc.tile_pool(name="stg", bufs=2) as stg, \
             tc.tile_pool(name="p2", bufs=2) as p2, \
             tc.tile_pool(name="p3", bufs=3) as p3, \
             tc.tile_pool(name="ps", bufs=6, space="PSUM") as psp, \
             tc.tile_pool(name="pst", bufs=2, space="PSUM") as pst:

        # ---------- constants ----------
            ident = cst.tile([P, P], f32, tag="ident")
            make_identity(nc, ident[:])
            ident_bf = cst.tile([P, P], bf16, tag="ident_bf")
            nc.vector.tensor_copy(ident_bf[:], ident[:])

            # fp8 weights (x32), contraction padded to 4 k-tiles of 128.
            def load_w8(name, dram_view, kchunks, width):
                """dram_view: [P, kchunks, width] f32 view; returns fp8
                [P, 4 or kchunks(+pad), width] tile with zero padding."""
                kalloc = 4 if kchunks == CC else kchunks
                w8 = cst.tile([P, kalloc, width], f8, tag=f"{name}8")
                if kalloc != kchunks:
                    nc.gpsimd.memset(w8[:, kchunks:kalloc], 0.0)
                st = stg.tile([P, kchunks, width], f32, tag="wstage",
                              name=f"{name}_stage")
                nc.sync.dma_start(st[:], dram_view)
                nc.scalar.activation(w8[:, 0:kchunks], st[:], AF.Copy, scale=WS)
                return w8

            Wq8 = load_w8("Wq", Wq_d.rearrange("h (o p) d -> p o (h d)", p=P), CC, C)
            Wk8 = load_w8("Wk", Wk_d.rearrange("h (o p) d -> p o (h d)", p=P), CC, C)
            Wv8 = load_w8("Wv", Wv_d.rearrange("h (o p) d -> p o (h d)", p=P), CC, C)
            Wp8 = load_w8("Wp", Wp_d.rearrange("(o p) c -> p o c", p=P), CC, C)
            W18 = load_w8("W1", W1_d.rearrange("(o p) f -> p o f", p=P), CC, FF)
            W28 = load_w8("W2", W2_d.rearrange("(o p) c -> p o c", p=P), FC, C)

            g1_sb = cst.tile([P, CC], f32, tag="g1")
            nc.sync.dma_start(g1_sb[:], ln1_g_d.rearrange("(o p) -> p o", p=P))
            lb1_sb = cst.tile([P, CC], f32, tag="lb1")
            nc.sync.dma_start(lb1_sb[:], ln1_b_d.rearrange("(o p) -> p o", p=P))
            g2_sb = cst.tile([P, CC], f32, tag="g2")
            nc.sync.dma_start(g2_sb[:], ln2_g_d.rearrange("(o p) -> p o", p=P))
            lb2_sb = cst.tile([P, CC], f32, tag="lb2")
            nc.sync.dma_start(lb2_sb[:], ln2_b_d.rearrange("(o p) -> p o", p=P))

            bq_sb = cst.tile([P, CC], f32, tag="bq")
            nc.sync.dma_start(
                bq_sb[:], bq_d.rearrange("h d -> (h d)").rearrange("(o p) -> p o", p=P))
            bk_sb = cst.tile([P, CC], f32, tag="bk")
            nc.sync.dma_start(
                bk_sb[:], bk_d.rearrange("h d -> (h d)").rearrange("(o p) -> p o", p=P))
            bp_sb = cst.tile([P, CC], f32, tag="bp")
            nc.sync.dma_start(bp_sb[:], bp_d.rearrange("(o p) -> p o", p=P))
            b1f_sb = cst.tile([P, FC], f32, tag="b1f")
            nc.sync.dma_start(b1f_sb[:], b1_d.rearrange("(o p) -> p o", p=P))
            b2_sb = cst.tile([P, CC], f32, tag="b2")
            nc.sync.dma_start(b2_sb[:], b2_d.rearrange("(o p) -> p o", p=P))

            # bv row (x32, fp8) + fp8 ones row for the V bias matmul
            bvst = cst.tile([1, C], f32, tag="bvst")
            nc.sync.dma_start(bvst[:], bv_d.rearrange("h d -> (h d)")[None])
            bvrow8 = cst.tile([1, C], f8, tag="bvrow8")
            nc.scalar.activation(bvrow8[:], bvst[:], AF.Copy, scale=WS)
            ones8 = cst.tile([1, P], f8, tag="ones8")
            nc.gpsimd.memset(ones8[:], 1.0)

            eps_sb = cst.tile([P, 1], f32, tag="eps")
            nc.gpsimd.memset(eps_sb[:], EPS)

            # sel2: rows 64/65 pick even/odd head reciprocal, value AVS
            sel2 = cst.tile([P, P], f32r, tag="sel2")
            nc.gpsimd.memset(sel2[64:66, :], 0.0)
            nc.gpsimd.memset(sel2[64:65, 0:64], AVS)
            nc.gpsimd.memset(sel2[65:66, 64:128], AVS)

            # additive causal mask for diagonal 128x128 blocks: 0 keep, -1e30 drop
            tri_neg = cst.tile([P, P], f32, tag="tri_neg")
            nc.gpsimd.memset(tri_neg[:], 0.0)
            nc.gpsimd.affine_select(
                out=tri_neg[:], in_=tri_neg[:],
                compare_op=OP.is_ge, fill=-1e30,
                base=0, pattern=[[1, P]], channel_multiplier=-1)

            # ---------- per-pair pools (pre-padded slots) ----------
            # V-hat fp8 [P, to(4), H, 128]: cols 0:64 V, col 64 ones (even
            # heads), col 65 ones (odd heads), rest zero. Pads written once
            # per slot; per-pair ACT copies touch only cols 0:64.
            V_slots = []
            for _ in range(2):
                V_sb = p2.tile([P, 4, H, P], f8, tag="V")
                nc.gpsimd.memset(V_sb[:, :, :, 64:128], 0.0)
                for h in range(H):
                    nc.gpsimd.memset(V_sb[:, :, h, 64 + (h % 2):65 + (h % 2)], 1.0)
                V_slots.append(V_sb)

            # E fp8 [P, sc(2), bb(2), 256]: sc=1 cols 0:128 permanently zero.
            for _ in range(3):
                E = p3.tile([P, 2, 2, 256], f8, tag="E")
                nc.gpsimd.memset(E[:, 1, :, 0:128], 0.0)

            # h1T/h2T/AVT fp8 [P, 4, TP]: 4th k-tile permanently zero.
            hT_slots = {}
            for tag in ("h1T", "h2T", "AVT"):
                hT_slots[tag] = []
                for _ in range(2):
                    t8 = p2.tile([P, 4, TP], f8, tag=tag)
                    nc.gpsimd.memset(t8[:, 3, :], 0.0)
                    hT_slots[tag].append(t8)

            def psum(tag="mm"):
                return psp.tile([P, TP], f32, tag=tag, name=tag)

            def psum_t():
                return pst.tile([P, TP], bf16, tag="tp", name="tp")

            # ---------- helpers ----------
            def layernorm(src_tok, g_sb, lb_sb, dstT, tagp, ht_engine):
                """src_tok [P,4,C] f32 -> dstT fp8 [P,4,TP] (k-tiles 0:3)."""
                st = p2.tile([P, 4, 6], f32, tag=f"{tagp}_st")
                mv = p2.tile([P, 4, 2], f32, tag=f"{tagp}_mv")
                for so in range(4):
                    nc.vector.bn_stats(st[:, so], src_tok[:, so])
                    nc.vector.bn_aggr(mv[:, so], st[:, so])
                sd = p2.tile([P, 4], f32, tag=f"{tagp}_sd")
                nc.scalar.activation(sd[:], mv[:, :, 1], AF.Sqrt, bias=eps_sb[:])
                rs = p2.tile([P, 4], f32, tag=f"{tagp}_rs")
                nc.vector.reciprocal(rs[:], sd[:])
                nmurs = p2.tile([P, 4], f32, tag=f"{tagp}_nmurs")
                # -mu * rs
                nc.vector.tensor_tensor(nmurs[:], mv[:, :, 0], rs[:], OP.mult)
                nc.vector.tensor_scalar_mul(nmurs[:], nmurs[:], -1.0)
                htok = p2.tile([P, 4, C], bf16, tag=f"{tagp}_htok")
                for so in range(4):
                    if ht_engine == "act":
                        nc.scalar.activation(
                            htok[:, so], src_tok[:, so], AF.Identity,
                            bias=nmurs[:, so:so + 1], scale=rs[:, so:so + 1])
                    else:
                        nc.vector.tensor_scalar(
                            htok[:, so], src_tok[:, so], rs[:, so:so + 1],
                            nmurs[:, so:so + 1], OP.mult, OP.add)
                for c in range(CC):
                    tp = psum_t()
                    for so in range(4):
                        nc.tensor.matmul(
                            tp[:, P * so:P * so + P],
                            htok[:, so, P * c:P * c + P].bitcast(bf16),
                            ident_bf[:], is_transpose=True)
                    nc.gpsimd.tensor_scalar(
                        dstT[:, c], tp[:], g_sb[:, c:c + 1], lb_sb[:, c:c + 1],
                        OP.mult, OP.add)

            def mm_c4(ps, W8t, xT, col, width=None):
                """psum[:, :width] += W8t[:, :, col:col+128].T @ xT over 4
                k-tiles via 2 DoubleRows."""
                for j in (0, 2):
                    nc.tensor.matmul(
                        ps if width is None else ps[:, 0:width],
                        W8t[:, j:j + 2, P * col:P * col + P],
                        xT[:, j:j + 2] if width is None else xT[:, j:j + 2, 0:width],
                        start=(j == 0), stop=(j == 2),
                        perf_mode=PM.DoubleRow)

            # ---------- per-pair loop ----------
            import contextlib
            rep_ctx = tc.For_i(0, repeat, 1) if repeat > 1 else contextlib.nullcontext()
            with rep_ctx:
              for pr in range(n_pairs):
                  x_view = x_d[2 * pr:2 * pr + 2].rearrange("b (o p) c -> p (b o) c", p=P)
                  y_view = y_d[2 * pr:2 * pr + 2].rearrange("b (o p) c -> p (b o) c", p=P)

                  x_tok = p2.tile([P, 4, C], f32, tag="x_tok")
                  nc.sync.dma_start(x_tok[:], x_view)

                  h1T = p2.tile([P, 4, TP], f8, tag="h1T")
                  layernorm(x_tok, g1_sb, lb1_sb, h1T, "ln1", "act")

                  # ---- Q^T, K^T fp8 ----
                  QT = p2.tile([P, CC, TP], f8, tag="QT")
                  KT = p2.tile([P, CC, TP], f8, tag="KT")
                  for (W8t, b_sb, dst) in ((Wq8, bq_sb, QT), (Wk8, bk_sb, KT)):
                      for mo in range(CC):
                          ps = psum()
                          mm_c4(ps[:], W8t, h1T, mo)
                          nc.scalar.activation(dst[:, mo], ps[:], AF.Identity,
                                               bias=b_sb[:, mo:mo + 1],
                                               scale=1.0 / WS)

                  # ---- V fp8 token-major (+bias row) ----
                  V_sb = p2.tile([P, 4, H, P], f8, tag="V")
                  for to in range(4):
                      ps = psum()
                      for j in (0, 2):
                          nc.tensor.matmul(
                              ps[:, 0:C],
                              h1T[:, j:j + 2, P * to:P * to + P],
                              Wv8[:, j:j + 2], start=(j == 0), stop=False,
                              perf_mode=PM.DoubleRow)
                      nc.tensor.matmul(ps[:, 0:C], ones8[:], bvrow8[:],
                                       start=False, stop=True)
                      nc.gpsimd.tensor_scalar(
                          V_sb[:, to, :, 0:64],
                          ps[:, 0:C].rearrange("p (h d) -> p h d", h=H),
                          1.0 / WS, None, OP.mult)

                  if debug_outputs and pr == 0:
                      dv = p2.tile([P, TP], f32, tag="dbgv")
                      nc.vector.tensor_copy(
                          dv[:], V_sb[:, 0].rearrange("p h d -> p (h d)")[:, 0:TP])
                      nc.sync.dma_start(dbg["V0"][:], dv[:])
                      dq = p2.tile([P, TP], f32, tag="dbgq")
                      nc.vector.tensor_copy(dq[:], QT[:, 0])
                      nc.sync.dma_start(dbg["QT0"][:], dq[:])
                      dk = p2.tile([P, TP], f32, tag="dbgk")
                      nc.vector.tensor_copy(dk[:], KT[:, 0])
                      nc.sync.dma_start(dbg["KT0"][:], dk[:])
                      dh = p2.tile([P, TP], f32, tag="dbgh")
                      nc.vector.tensor_copy(dh[:], h1T[:, 0])
                      nc.sync.dma_start(dbg["h1T0"][:], dh[:])

                  # ---- attention ----
                  AVT = p2.tile([P, 4, TP], f8, tag="AVT")
                  for h in range(H):
                      mo, half = h // 2, h % 2
                      rows = slice(64 * half, 64 * half + 64)
                      if half == 0:
                          den = p2.tile([P, TP], f32r, tag="den")
                          rec = p2.tile([P, TP], f32r, tag="rec")
                      # scores^T: rows of sps = s (softmax dim), cols = (bb, t)
                      sps0 = psum()
                      sps1 = psum()
                      for bb in range(2):
                          nc.tensor.matmul(
                              sps0[:, 256 * bb:256 * bb + 256],
                              QT[rows, mo, 256 * bb:256 * bb + 128],
                              KT[rows, mo, 256 * bb:256 * bb + 256],
                              start=True, stop=True)
                          nc.tensor.matmul(
                              sps1[:, 128 * bb:128 * bb + 128],
                              QT[rows, mo, 256 * bb + 128:256 * bb + 256],
                              KT[rows, mo, 256 * bb + 128:256 * bb + 256],
                              start=True, stop=True)
                      # additive causal mask on the diagonal blocks
                      nc.vector.tensor_tensor(
                          sps0[:].rearrange("p (b t) -> p b t", b=2)[:, :, 0:128],
                          sps0[:].rearrange("p (b t) -> p b t", b=2)[:, :, 0:128],
                          tri_neg[:, None, :].to_broadcast((P, 2, P)), OP.add)
                      nc.vector.tensor_tensor(
                          sps1[:, 0:256].rearrange("p (b t) -> p b t", b=2),
                          sps1[:, 0:256].rearrange("p (b t) -> p b t", b=2),
                          tri_neg[:, None, :].to_broadcast((P, 2, P)), OP.add)
                      E = p3.tile([P, 2, 2, 256], f8, tag="E")
                      nc.scalar.activation(
                          E[:, 0], sps0[:].rearrange("p (b t) -> p b t", b=2),
                          AF.Exp, scale=SCALE)
                      nc.scalar.activation(
                          E[:, 1, :, 128:256],
                          sps1[:, 0:256].rearrange("p (b t) -> p b t", b=2),
                          AF.Exp, scale=SCALE)
                      if debug_outputs and pr == 0 and h == 0:
                          de = p2.tile([P, TP], f32, tag="dbge")
                          nc.vector.tensor_copy(
                              de[:], E[:, 0].rearrange("p b t -> p (b t)"))
                          nc.sync.dma_start(dbg["E0"][:], de[:])
                      avps = psum()
                      for bb in range(2):
                          nc.tensor.matmul(
                              avps[0:66, 256 * bb:256 * bb + 256],
                              V_sb[:, 2 * bb:2 * bb + 2, h, 0:66],
                              E[:, :, bb, :],
                              start=True, stop=True, perf_mode=PM.DoubleRow)
                      # denominator row (64 even / 65 odd) -> rec tile
                      r = 64 + half
                      nc.scalar.activation(den[r:r + 1, :].bitcast(f32),
                                           avps[r:r + 1, :], AF.Copy)
                      if half == 1:
                          with nc.allow_low_precision(reason="softmax recip"):
                              nc.vector.reciprocal(rec[64:66, :], den[64:66, :])
                          rps2 = psum()
                          nc.tensor.matmul(rps2[:], sel2[64:66, :], rec[64:66, :],
                                           start=True, stop=True)
                          for half2 in range(2):
                              rows2 = slice(64 * half2, 64 * half2 + 64)
                              nc.vector.tensor_tensor(
                                  AVT[rows2, mo], avs_list[half2][0:64, :],
                                  rps2[rows2, :], OP.mult)
                          avs_list = []
                      if half == 0:
                          avs_list = [avps]
                      else:
                          pass
                      if half == 0:
                          pass
                      else:
                          pass
                      if half == 0:
                          avs_keep = avps
                      # (avs_list managed above)

                  # ---- proj + residual ----
                  proj_sb = p2.tile([P, CC, TP], bf16, tag="proj_sb")
                  for mo in range(CC):
                      ps = psum()
                      mm_c4(ps[:], Wp8, AVT, mo)
                      nc.scalar.activation(proj_sb[:, mo], ps[:], AF.Identity,
                                           bias=bp_sb[:, mo:mo + 1],
                                           scale=1.0 / (WS * AVS))
                  if debug_outputs and pr == 0:
                      dp = p2.tile([P, TP], f32, tag="dbgp")
                      nc.vector.tensor_copy(dp[:], proj_sb[:, 0])
                      nc.sync.dma_start(dbg["proj0"][:], dp[:])
                      da = p2.tile([P, TP], f32, tag="dbga")
                      nc.vector.tensor_copy(da[:], AVT[:, 0])
                      nc.sync.dma_start(dbg["AVT0"][:], da[:])
                  out1_tok = p2.tile([P, 4, C], f32, tag="out1_tok")
                  for so in range(4):
                      tp = psum_t()
                      for mo in range(CC):
                          nc.tensor.matmul(
                              tp[:, P * mo:P * mo + P],
                              proj_sb[:, mo, P * so:P * so + P].bitcast(bf16),
                              ident_bf[:], is_transpose=True)
                      nc.vector.tensor_tensor(out1_tok[:, so], tp[:, 0:C],
                                              x_tok[:, so], OP.add)

                  if debug_outputs and pr == 0:
                      do1 = p2.tile([P, 4 * C], f32, tag="dbgo")
                      nc.vector.tensor_copy(
                          do1[:], out1_tok[:].rearrange("p a c -> p (a c)"))
                      nc.sync.dma_start(dbg["out1"][:], do1[:])

                  # ---- LN2 + FFN ----
                  h2T = p2.tile([P, 4, TP], f8, tag="h2T")
                  layernorm(out1_tok, g2_sb, lb2_sb, h2T, "ln2", "dve")

                  FF_sb = p2.tile([P, FC, TP], f8, tag="FF_sb")
                  for fo in range(FC):
                      ps = psum()
                      mm_c4(ps[:], W18, h2T, fo)
                      nc.scalar.activation(FF_sb[:, fo], ps[:], AF.Relu,
                                           bias=b1f_sb[:, fo:fo + 1],
                                           scale=1.0 / WS)
                  if debug_outputs and pr == 0:
                      df = p2.tile([P, TP], f32, tag="dbgf")
                      nc.vector.tensor_copy(df[:], FF_sb[:, 0])
                      nc.sync.dma_start(dbg["FF0"][:], df[:])
                      dh2 = p2.tile([P, TP], f32, tag="dbgh2")
                      nc.vector.tensor_copy(dh2[:], h2T[:, 0])
                      nc.sync.dma_start(dbg["h2T0"][:], dh2[:])

                  g_sb = p2.tile([P, CC, TP], bf16, tag="g_sb")
                  for mo in range(CC):
                      ps = psum()
                      for j in range(0, FC, 2):
                          nc.tensor.matmul(
                              ps[:], W28[:, j:j + 2, P * mo:P * mo + P],
                              FF_sb[:, j:j + 2], start=(j == 0),
                              stop=(j == FC - 2), perf_mode=PM.DoubleRow)
                      nc.scalar.activation(g_sb[:, mo], ps[:], AF.Identity,
                                           bias=b2_sb[:, mo:mo + 1],
                                           scale=1.0 / WS)

                  y_tok = p2.tile([P, 4, C], f32, tag="y_tok")
                  for so in range(4):
                      tp = psum_t()
                      for mo in range(CC):
                          nc.tensor.matmul(
                              tp[:, P * mo:P * mo + P],
                              g_sb[:, mo, P * so:P * so + P].bitcast(bf16),
                              ident_bf[:], is_transpose=True)
                      nc.vector.tensor_tensor(y_tok[:, so], tp[:, 0:C],
                                              out1_tok[:, so], OP.add)
                  nc.sync.dma_start(y_view, y_tok[:])

    nc.compile()
    return nc


_NC_CACHE = {}


def kernel(_run_kwargs=None, **inputs) -> np.ndarray:
    run_kwargs = _run_kwargs or {}
    x = np.ascontiguousarray(np.asarray(inputs["x"], dtype=np.float32))
    weights = {k: np.ascontiguousarray(np.asarray(v, dtype=np.float32))
               for k, v in inputs.items() if k != "x"}

    if "nc" not in _NC_CACHE:
        _NC_CACHE["nc"] = build_nc()
    nc = _NC_CACHE["nc"]

    in_maps = []
    for c in range(N_CORES):
        m = {"x": x[c * B_LOCAL:(c + 1) * B_LOCAL]}
        m.update(weights)
        in_maps.append(m)

    res = run_bass_kernel_spmd(nc, in_maps, core_ids=list(range(N_CORES)), **run_kwargs)
    y = np.concatenate([r["y"] for r in res.results], axis=0)
    kernel.last_result = res
    return y


# revision 17
# speedup vs baseline: 1.3862x; 1.3862x over previous
"""Trainium2 Bass kernel for nn_Block_82111184765408 (pre-LN transformer block).

B=128, T=256, C=384, H=6, D=64, FF=1536. Data-parallel over batch across 8
NeuronCores (16 batches/core), batches processed in fused pairs (free dim 512).

v2: fp8(e4m3) DoubleRow matmuls for all C/FF contractions (weights scaled x32
on-device), bf16 transposes, bn_stats layernorm, causal-skip scores/exp,
additive -inf pre-masking on psum, parity-column softmax denominators with a
shared per-mo reciprocal broadcast, and double-buffered tiles so consecutive
batch pairs pipeline across engines.

Dataflow per pair: x (token-major f32) -> bn_stats LN1 -> htok bf16 ->
PE-transpose (bf16 identity) -> h1T fp8 [P,4,TP] (4th k-tile zero-padded,
gain/bias folded) -> QT/KT fp8 (DoubleRow) / V fp8 token-major (DoubleRow,
bias via ones-row matmul, parity ones cols 64/65 for denominators) ->
scoresT per head (causal-skipped, -1e30 premask on psum) -> exp fp8 ->
attn@V DoubleRow (denom rides in rows 64/65) -> per-mo reciprocal +
sel-matmul broadcast (x8 for fp8 range) -> AVT fp8 -> proj (DoubleRow) ->
transpose-back + residual f32 -> LN2 -> FFN (DoubleRow both layers, relu on
ACT) -> transpose + residual -> DMA out.
"""

import numpy as np

import concourse.bass as bass
import concourse.mybir as mybir
import concourse.tile as tile
from concourse import bacc
from concourse.bass_utils import run_bass_kernel_spmd
from concourse.masks import make_identity

P = 128
B, T, C, H, D = 128, 256, 384, 6, 64
FF = 4 * C
N_CORES = 8
B_LOCAL = B // N_CORES          # 16 batches per core
N_PAIRS = B_LOCAL // 2          # 8 pairs, free dim 512 per pair
TP = 2 * T                      # 512
CC = C // P                     # 3 feature chunks
FC = FF // P                    # 12 ffn chunks
EPS = 1e-5
SCALE = C ** -0.5
WS = 32.0                       # fp8 weight scale
AVS = 8.0                       # AVT scale (baked into sel2)

f32 = mybir.dt.float32
f32r = mybir.dt.float32r
bf16 = mybir.dt.bfloat16
f8 = mybir.dt.float8e4
AF = mybir.ActivationFunctionType
OP = mybir.AluOpType
PM = mybir.MatmulPerfMode


def build_nc(n_pairs=N_PAIRS, debug_outputs=False, repeat=1):
    nc = bacc.Bacc("TRN2", target_bir_lowering=False, debug=False)

    x_d = nc.declare_dram_parameter("x", [2 * n_pairs, T, C], f32, isOutput=False)
    ln1_g_d = nc.declare_dram_parameter("ln1_g", [C], f32, isOutput=False)
    ln1_b_d = nc.declare_dram_parameter("ln1_b", [C], f32, isOutput=False)
    bk_d = nc.declare_dram_parameter("bk", [H, D], f32, isOutput=False)
    bq_d = nc.declare_dram_parameter("bq", [H, D], f32, isOutput=False)
    bp_d = nc.declare_dram_parameter("bp", [C], f32, isOutput=False)
    ln2_g_d = nc.declare_dram_parameter("ln2_g", [C], f32, isOutput=False)
    ln2_b_d = nc.declare_dram_parameter("ln2_b", [C], f32, isOutput=False)
    b1_d = nc.declare_dram_parameter("b1", [FF], f32, isOutput=False)
    b2_d = nc.declare_dram_parameter("b2", [C], f32, isOutput=False)
    # host-prepared fp8 weights (x32, [P, ktiles, width], zero-padded)
    Wq8_d = nc.declare_dram_parameter("Wq8", [P, 4, C], f8, isOutput=False)
    Wk8_d = nc.declare_dram_parameter("Wk8", [P, 4, C], f8, isOutput=False)
    Wv8_d = nc.declare_dram_parameter("Wv8", [P, 4, C], f8, isOutput=False)
    Wp8_d = nc.declare_dram_parameter("Wp8", [P, 4, C], f8, isOutput=False)
    W18_d = nc.declare_dram_parameter("W18", [P, 4, FF], f8, isOutput=False)
    W28_d = nc.declare_dram_parameter("W28", [P, FC, C], f8, isOutput=False)
    bvrow8_d = nc.declare_dram_parameter("bvrow8", [1, C], f8, isOutput=False)
    ones8_d = nc.declare_dram_parameter("ones8", [1, P], f8, isOutput=False)
    sel2_d = nc.declare_dram_parameter("sel2", [P, P], f32r, isOutput=False)
    tri01_d = nc.declare_dram_parameter("tri01", [P, P], f32, isOutput=False)
    y_d = nc.declare_dram_parameter("y", [2 * n_pairs, T, C], f32, isOutput=True)
    dbg = {}
    if debug_outputs:
        for nm, shp in (("h1T0", [P, TP]), ("QT0", [P, TP]), ("KT0", [P, TP]),
                        ("V0", [P, TP]), ("E0", [P, TP]), ("AVT0", [P, TP]),
                        ("proj0", [P, TP]), ("out1", [P, 4 * C]),
                        ("h2T0", [P, TP]), ("FF0", [P, TP])):
            dbg[nm] = nc.declare_dram_parameter(nm, shp, f32, isOutput=True)

    with tile.TileContext(nc) as tc:
        with tc.tile_pool(name="const", bufs=1) as cst, \
             tc.tile_pool(name="p2", bufs=2) as p2, \
             tc.tile_pool(name="p3", bufs=3) as p3, \
             tc.tile_pool(name="ps", bufs=6, space="PSUM") as psp, \
             tc.tile_pool(name="pst", bufs=2, space="PSUM") as pst:

        # ---------- constants ----------
            ident = cst.tile([P, P], f32, tag="ident")
            make_identity(nc, ident[:])
            ident_bf = cst.tile([P, P], bf16, tag="ident_bf")
            nc.vector.tensor_copy(ident_bf[:], ident[:])

            # host-prepared fp8 weights: plain DMAs
            def load_w8(name, dram, kt, width):
                w8 = cst.tile([P, kt, width], f8, tag=f"{name}8")
                nc.sync.dma_start(w8[:], dram[:])
                return w8

            Wq8 = load_w8("Wq", Wq8_d, 4, C)
            Wk8 = load_w8("Wk", Wk8_d, 4, C)
            Wv8 = load_w8("Wv", Wv8_d, 4, C)
            Wp8 = load_w8("Wp", Wp8_d, 4, C)
            W18 = load_w8("W1", W18_d, 4, FF)
            W28 = load_w8("W2", W28_d, FC, C)

            g1_sb = cst.tile([P, CC], f32, tag="g1")
            nc.sync.dma_start(g1_sb[:], ln1_g_d.rearrange("(o p) -> p o", p=P))
            lb1_sb = cst.tile([P, CC], f32, tag="lb1")
            nc.sync.dma_start(lb1_sb[:], ln1_b_d.rearrange("(o p) -> p o", p=P))
            g2_sb = cst.tile([P, CC], f32, tag="g2")
            nc.sync.dma_start(g2_sb[:], ln2_g_d.rearrange("(o p) -> p o", p=P))
            lb2_sb = cst.tile([P, CC], f32, tag="lb2")
            nc.sync.dma_start(lb2_sb[:], ln2_b_d.rearrange("(o p) -> p o", p=P))

            bq_sb = cst.tile([P, CC], f32, tag="bq")
            nc.sync.dma_start(
                bq_sb[:], bq_d.rearrange("h d -> (h d)").rearrange("(o p) -> p o", p=P))
            bk_sb = cst.tile([P, CC], f32, tag="bk")
            nc.sync.dma_start(
                bk_sb[:], bk_d.rearrange("h d -> (h d)").rearrange("(o p) -> p o", p=P))
            bp_sb = cst.tile([P, CC], f32, tag="bp")
            nc.sync.dma_start(bp_sb[:], bp_d.rearrange("(o p) -> p o", p=P))
            b1f_sb = cst.tile([P, FC], f32, tag="b1f")
            nc.sync.dma_start(b1f_sb[:], b1_d.rearrange("(o p) -> p o", p=P))
            b2_sb = cst.tile([P, CC], f32, tag="b2")
            nc.sync.dma_start(b2_sb[:], b2_d.rearrange("(o p) -> p o", p=P))

            # bv row (x32, fp8) + fp8 ones row for the V bias matmul
            bvrow8 = cst.tile([1, C], f8, tag="bvrow8")
            nc.sync.dma_start(bvrow8[:], bvrow8_d[:])
            ones8 = cst.tile([1, P], f8, tag="ones8")
            nc.sync.dma_start(ones8[:], ones8_d[:])

            eps_sb = cst.tile([P, 1], f32, tag="eps")
            nc.gpsimd.memset(eps_sb[:], EPS)

            # sel2: row 64 -> AVS on cols 0:64 (even head), row 96 -> AVS on
            # cols 64:128 (odd head); rows 65:96 zero (host-prepared).
            sel2 = cst.tile([P, P], f32r, tag="sel2")
            nc.sync.dma_start(sel2[:], sel2_d[:])

            # multiplicative causal mask for diagonal 128x128 blocks
            tri01 = cst.tile([P, P], f32, tag="tri01")
            nc.sync.dma_start(tri01[:], tri01_d[:])

            # ---------- per-pair pools (pre-padded slots) ----------
            # V-hat fp8 [P, to(4), H, 128]: cols 0:64 V, col 64 ones (even
            # heads), col 65 ones (odd heads), rest zero. Pads written once
            # per slot; per-pair ACT copies touch only cols 0:64.
            V_slots = []
            for _ in range(2):
                V_sb = p2.tile([P, 4, H, P], f8, tag="V")
                nc.gpsimd.memset(V_sb[:, :, :, 64:128], 0.0)
                for h in range(H):
                    col = 64 if h % 2 == 0 else 96
                    nc.gpsimd.memset(V_sb[:, :, h, col:col + 1], 1.0)
                V_slots.append(V_sb)

            # E fp8 [P, sc(2), bb(2), 256]: sc=1 cols 0:128 permanently zero.
            E_slots = []
            for _ in range(3):
                E = p3.tile([P, 2, 2, 256], f8, tag="E")
                nc.gpsimd.memset(E[:, 1, :, 0:128], 0.0)
                E_slots.append(E)
            e_ctr = [0]

            def next_E():
                E = E_slots[e_ctr[0] % 3]
                e_ctr[0] += 1
                return E

            # rec slots: rows 64/96 written per mo-group; 65:96 stay zero
            rec_slots = []
            for _ in range(2):
                rc = p2.tile([P, TP], f32r, tag="rec")
                nc.vector.tensor_scalar(
                    rc[:].rearrange("p (a b) -> p a b", a=4),
                    tri01[:, None, :].to_broadcast((P, 4, P)),
                    0.0, 0.0, OP.mult, OP.add)
                rec_slots.append(rc)
            rec_ctr = [0]

            def next_rec():
                rc = rec_slots[rec_ctr[0] % 2]
                rec_ctr[0] += 1
                return rc

            # h1T/h2T/AVT fp8 [P, 4, TP]: 4th k-tile permanently zero.
            hT_slots = {}
            for tag in ("h1T", "h2T", "AVT"):
                hT_slots[tag] = []
                for _ in range(2):
                    t8 = p2.tile([P, 4, TP], f8, tag=tag)
                    nc.gpsimd.memset(t8[:, 3, :], 0.0)
                    hT_slots[tag].append(t8)

            def psum(tag="mm"):
                return psp.tile([P, TP], f32, tag=tag, name=tag)

            def psum_t():
                return pst.tile([P, TP], bf16, tag="tp", name="tp")

            # ---------- helpers ----------
            def layernorm(src_tok, g_sb, lb_sb, dstT, tagp, ht_engine):
                """src_tok [P,4,C] f32 -> dstT fp8 [P,4,TP] (k-tiles 0:3)."""
                st = p2.tile([P, 4, 6], f32, tag=f"{tagp}_st")
                mv = p2.tile([P, 4, 2], f32, tag=f"{tagp}_mv")
                for so in range(4):
                    nc.vector.bn_stats(st[:, so], src_tok[:, so])
                    nc.vector.bn_aggr(mv[:, so], st[:, so])
                sd = p2.tile([P, 4], f32, tag=f"{tagp}_sd")
                nc.scalar.activation(sd[:], mv[:, :, 1], AF.Sqrt, bias=eps_sb[:])
                rs = p2.tile([P, 4], f32, tag=f"{tagp}_rs")
                nc.vector.reciprocal(rs[:], sd[:])
                nmurs = p2.tile([P, 4], f32, tag=f"{tagp}_nmurs")
                # -mu * rs
                nc.vector.tensor_tensor(nmurs[:], mv[:, :, 0], rs[:], OP.mult)
                nc.vector.tensor_scalar_mul(nmurs[:], nmurs[:], -1.0)
                htok = p2.tile([P, 4, C], bf16, tag=f"{tagp}_htok")
                for so in range(4):
                    if ht_engine == "act":
                        nc.scalar.activation(
                            htok[:, so], src_tok[:, so], AF.Identity,
                            bias=nmurs[:, so:so + 1], scale=rs[:, so:so + 1])
                    else:
                        nc.vector.tensor_scalar(
                            htok[:, so], src_tok[:, so], rs[:, so:so + 1],
                            nmurs[:, so:so + 1], OP.mult, OP.add)
                for c in range(CC):
                    tp = psum_t()
                    for so in range(4):
                        nc.tensor.matmul(
                            tp[:, P * so:P * so + P],
                            htok[:, so, P * c:P * c + P].bitcast(bf16),
                            ident_bf[:], is_transpose=True)
                    if ht_engine == "act":
                        nc.vector.tensor_scalar(
                            dstT[:, c], tp[:], g_sb[:, c:c + 1], lb_sb[:, c:c + 1],
                            OP.mult, OP.add)
                    else:
                        nc.scalar.activation(
                            dstT[:, c], tp[:], AF.Identity,
                            bias=lb_sb[:, c:c + 1], scale=g_sb[:, c:c + 1])

            def mm_c4(ps, W8t, xT, col, width=None):
                """psum[:, :width] += W8t[:, :, col:col+128].T @ xT over 4
                k-tiles via 2 DoubleRows."""
                for j in (0, 2):
                    nc.tensor.matmul(
                        ps if width is None else ps[:, 0:width],
                        W8t[:, j:j + 2, P * col:P * col + P],
                        xT[:, j:j + 2] if width is None else xT[:, j:j + 2, 0:width],
                        start=(j == 0), stop=(j == 2),
                        perf_mode=PM.DoubleRow)

            # ---------- per-pair loop ----------
            import contextlib
            rep_ctx = tc.For_i(0, repeat, 1) if repeat > 1 else contextlib.nullcontext()
            with rep_ctx:
              for pr in range(n_pairs):
                  x_view = x_d[2 * pr:2 * pr + 2].rearrange("b (o p) c -> p (b o) c", p=P)
                  y_view = y_d[2 * pr:2 * pr + 2].rearrange("b (o p) c -> p (b o) c", p=P)

                  x_tok = p2.tile([P, 4, C], f32, tag="x_tok")
                  nc.sync.dma_start(x_tok[:], x_view)

                  h1T = hT_slots["h1T"][pr % 2]
                  layernorm(x_tok, g1_sb, lb1_sb, h1T, "ln1", "act")

                  # ---- Q^T, K^T fp8 ----
                  QT = p2.tile([P, CC, TP], f8, tag="QT")
                  KT = p2.tile([P, CC, TP], f8, tag="KT")
                  for (W8t, b_sb, dst) in ((Wq8, bq_sb, QT), (Wk8, bk_sb, KT)):
                      for mo in range(CC):
                          ps = psum()
                          mm_c4(ps[:], W8t, h1T, mo)
                          if dst is QT:
                              nc.scalar.activation(dst[:, mo], ps[:], AF.Identity,
                                                   bias=b_sb[:, mo:mo + 1],
                                                   scale=1.0 / WS)
                          else:
                              nc.vector.tensor_scalar(
                                  dst[:, mo], ps[:], 1.0 / WS,
                                  b_sb[:, mo:mo + 1], OP.mult, OP.add)

                  # ---- V fp8 token-major (+bias row) ----
                  V_sb = V_slots[pr % 2]
                  for to in range(4):
                      ps = psum()
                      for j in (0, 2):
                          nc.tensor.matmul(
                              ps[:, 0:C],
                              h1T[:, j:j + 2, P * to:P * to + P],
                              Wv8[:, j:j + 2], start=(j == 0), stop=False,
                              perf_mode=PM.DoubleRow)
                      nc.tensor.matmul(ps[:, 0:C], ones8[:], bvrow8[:],
                                       start=False, stop=True)
                      nc.scalar.activation(
                          V_sb[:, to, :, 0:64],
                          ps[:, 0:C].rearrange("p (h d) -> p h d", h=H),
                          AF.Copy, scale=1.0 / WS)

                  if debug_outputs and pr == 0:
                      dv = p2.tile([P, TP], f32, tag="dbgv")
                      nc.vector.tensor_copy(
                          dv[:], V_sb[:, 0].rearrange("p h d -> p (h d)")[:, 0:TP])
                      nc.sync.dma_start(dbg["V0"][:], dv[:])
                      dq = p2.tile([P, TP], f32, tag="dbgq")
                      nc.vector.tensor_copy(dq[:], QT[:, 0])
                      nc.sync.dma_start(dbg["QT0"][:], dq[:])
                      dk = p2.tile([P, TP], f32, tag="dbgk")
                      nc.vector.tensor_copy(dk[:], KT[:, 0])
                      nc.sync.dma_start(dbg["KT0"][:], dk[:])
                      dh = p2.tile([P, TP], f32, tag="dbgh")
                      nc.vector.tensor_copy(dh[:], h1T[:, 0])
                      nc.sync.dma_start(dbg["h1T0"][:], dh[:])

                  # ---- attention ----
                  AVT = hT_slots["AVT"][pr % 2]
                  for mo in range(CC):
                      rec = next_rec()
                      av_pair = {}
                      for half in range(2):
                          h = 2 * mo + half
                          rows = slice(64 * half, 64 * half + 64)
                          # scores^T: sps rows = s (softmax dim), cols = (bb, t)
                          sps0 = psum()
                          sps1 = psum()
                          for bb in range(2):
                              nc.tensor.matmul(
                                  sps0[:, 256 * bb:256 * bb + 256],
                                  QT[rows, mo, 256 * bb:256 * bb + 128],
                                  KT[rows, mo, 256 * bb:256 * bb + 256],
                                  start=True, stop=True)
                              nc.tensor.matmul(
                                  sps1[:, 128 * bb:128 * bb + 128],
                                  QT[rows, mo, 256 * bb + 128:256 * bb + 256],
                                  KT[rows, mo, 256 * bb + 128:256 * bb + 256],
                                  start=True, stop=True)
                          E = next_E()
                          nc.scalar.activation(
                              E[:, 0], sps0[:].rearrange("p (b t) -> p b t", b=2),
                              AF.Exp, scale=SCALE)
                          nc.scalar.activation(
                              E[:, 1, :, 128:256],
                              sps1[:, 0:256].rearrange("p (b t) -> p b t", b=2),
                              AF.Exp, scale=SCALE)
                          # multiplicative causal mask on diagonal blocks (Pool)
                          nc.gpsimd.tensor_tensor(
                              E[:, 0, :, 0:128], E[:, 0, :, 0:128],
                              tri01[:, None, :].to_broadcast((P, 2, P)), OP.mult)
                          nc.gpsimd.tensor_tensor(
                              E[:, 1, :, 128:256], E[:, 1, :, 128:256],
                              tri01[:, None, :].to_broadcast((P, 2, P)), OP.mult)
                          if debug_outputs and pr == 0 and h == 0:
                              de = p2.tile([P, TP], f32, tag="dbge")
                              nc.vector.tensor_copy(
                                  de[:], E[:, 0].rearrange("p b t -> p (b t)"))
                              nc.sync.dma_start(dbg["E0"][:], de[:])
                          avps = psum()
                          for bb in range(2):
                              nc.tensor.matmul(
                                  avps[0:97, 256 * bb:256 * bb + 256],
                                  V_sb[:, 2 * bb:2 * bb + 2, h, 0:97],
                                  E[:, :, bb, :],
                                  start=True, stop=True, perf_mode=PM.DoubleRow)
                          av_pair[half] = avps
                          # reciprocal of denominator row, straight from psum
                          r = 64 if half == 0 else 96
                          with nc.allow_low_precision(reason="softmax recip"):
                              nc.vector.reciprocal(rec[r:r + 1, :],
                                                   avps[r:r + 1, :])
                      rps2 = psum()
                      nc.tensor.matmul(rps2[:], sel2[64:97, :], rec[64:97, :],
                                       start=True, stop=True)
                      rps_sb = p2.tile([P, TP], bf16, tag="rps_sb")
                      nc.scalar.activation(rps_sb[:], rps2[:], AF.Copy)
                      for half in range(2):
                          rows = slice(64 * half, 64 * half + 64)
                          nc.vector.tensor_tensor(
                              AVT[rows, mo], av_pair[half][0:64, :],
                              rps_sb[rows, :], OP.mult)

                  # ---- proj + residual ----
                  proj_sb = p2.tile([P, CC, TP], bf16, tag="proj_sb")
                  for mo in range(CC):
                      ps = psum()
                      mm_c4(ps[:], Wp8, AVT, mo)
                      nc.scalar.activation(proj_sb[:, mo], ps[:], AF.Identity,
                                           bias=bp_sb[:, mo:mo + 1],
                                           scale=1.0 / (WS * AVS))
                  if debug_outputs and pr == 0:
                      dp = p2.tile([P, TP], f32, tag="dbgp")
                      nc.vector.tensor_copy(dp[:], proj_sb[:, 0])
                      nc.sync.dma_start(dbg["proj0"][:], dp[:])
                      da = p2.tile([P, TP], f32, tag="dbga")
                      nc.vector.tensor_copy(da[:], AVT[:, 0])
                      nc.sync.dma_start(dbg["AVT0"][:], da[:])
                  out1_tok = p2.tile([P, 4, C], f32, tag="out1_tok")
                  for so in range(4):
                      tp = psum_t()
                      for mo in range(CC):
                          nc.tensor.matmul(
                              tp[:, P * mo:P * mo + P],
                              proj_sb[:, mo, P * so:P * so + P].bitcast(bf16),
                              ident_bf[:], is_transpose=True)
                      nc.vector.tensor_tensor(out1_tok[:, so], tp[:, 0:C],
                                              x_tok[:, so], OP.add)

                  if debug_outputs and pr == 0:
                      do1 = p2.tile([P, 4 * C], f32, tag="dbgo")
                      nc.vector.tensor_copy(
                          do1[:], out1_tok[:].rearrange("p a c -> p (a c)"))
                      nc.sync.dma_start(dbg["out1"][:], do1[:])

                  # ---- LN2 + FFN ----
                  h2T = hT_slots["h2T"][pr % 2]
                  layernorm(out1_tok, g2_sb, lb2_sb, h2T, "ln2", "dve")

                  FF_sb = p2.tile([P, FC, TP], f8, tag="FF_sb")
                  for fo in range(FC):
                      ps = psum()
                      mm_c4(ps[:], W18, h2T, fo)
                      nc.scalar.activation(FF_sb[:, fo], ps[:], AF.Relu,
                                           bias=b1f_sb[:, fo:fo + 1],
                                           scale=1.0 / WS)
                  if debug_outputs and pr == 0:
                      df = p2.tile([P, TP], f32, tag="dbgf")
                      nc.vector.tensor_copy(df[:], FF_sb[:, 0])
                      nc.sync.dma_start(dbg["FF0"][:], df[:])
                      dh2 = p2.tile([P, TP], f32, tag="dbgh2")
                      nc.vector.tensor_copy(dh2[:], h2T[:, 0])
                      nc.sync.dma_start(dbg["h2T0"][:], dh2[:])

                  g_sb = p2.tile([P, CC, TP], bf16, tag="g_sb")
                  for mo in range(CC):
                      ps = psum()
                      for j in range(0, FC, 2):
                          nc.tensor.matmul(
                              ps[:], W28[:, j:j + 2, P * mo:P * mo + P],
                              FF_sb[:, j:j + 2], start=(j == 0),
                              stop=(j == FC - 2), perf_mode=PM.DoubleRow)
                      nc.scalar.activation(g_sb[:, mo], ps[:], AF.Identity,
                                           bias=b2_sb[:, mo:mo + 1],
                                           scale=1.0 / WS)

                  y_tok = p2.tile([P, 4, C], f32, tag="y_tok")
                  for so in range(4):
                      tp = psum_t()
                      for mo in range(CC):
                          nc.tensor.matmul(
                              tp[:, P * mo:P * mo + P],
                              g_sb[:, mo, P * so:P * so + P].bitcast(bf16),
                              ident_bf[:], is_transpose=True)
                      nc.vector.tensor_tensor(y_tok[:, so], tp[:, 0:C],
                                              out1_tok[:, so], OP.add)
                  nc.sync.dma_start(y_view, y_tok[:])

    nc.compile()
    return nc


_NC_CACHE = {}


def prep_inputs(inputs):
    """Host-side prep: fp8(x32) weights in [P, ktiles, width] layout plus
    sel2/tri01/ones constants. Returns the non-x input map."""
    import ml_dtypes
    e4m3 = ml_dtypes.float8_e4m3

    def f(k):
        return np.ascontiguousarray(np.asarray(inputs[k], dtype=np.float32))

    def q8(a):
        return np.ascontiguousarray((a * WS).astype(e4m3))

    def chunked(w, width):  # [C_in, width] -> [P, 4, width] padded fp8
        arr = np.zeros((P, 4, width), np.float32)
        arr[:, 0:CC] = w.reshape(CC, P, width).transpose(1, 0, 2)
        return q8(arr)

    Wq, Wk, Wv = f("Wq"), f("Wk"), f("Wv")
    qkv = {}
    for nm, W in (("Wq8", Wq), ("Wk8", Wk), ("Wv8", Wv)):
        arr = np.zeros((P, 4, C), np.float32)
        for h in range(H):
            arr[:, 0:CC, 64 * h:64 * h + 64] = (
                W[h].reshape(CC, P, D).transpose(1, 0, 2))
        qkv[nm] = q8(arr)

    sel2 = np.zeros((P, P), np.float32)
    sel2[64, 0:64] = AVS
    sel2[96, 64:128] = AVS
    tri01 = np.triu(np.ones((P, P), np.float32))

    m = {
        "ln1_g": f("ln1_g"), "ln1_b": f("ln1_b"),
        "ln2_g": f("ln2_g"), "ln2_b": f("ln2_b"),
        "bq": f("bq"), "bk": f("bk"), "bp": f("bp"),
        "b1": f("b1"), "b2": f("b2"),
        "Wp8": chunked(f("Wp"), C),
        "W18": chunked(f("W1"), FF),
        "W28": np.ascontiguousarray(
            (f("W2").reshape(FC, P, C).transpose(1, 0, 2) * WS).astype(e4m3)),
        "bvrow8": q8(f("bv").reshape(1, -1)),
        "ones8": np.ones((1, P), e4m3),
        "sel2": sel2, "tri01": tri01,
    }
    m.update(qkv)
    return m


def kernel(_run_kwargs=None, **inputs) -> np.ndarray:
    run_kwargs = _run_kwargs or {}
    x = np.ascontiguousarray(np.asarray(inputs["x"], dtype=np.float32))
    weights = prep_inputs(inputs)

    if "nc" not in _NC_CACHE:
        _NC_CACHE["nc"] = build_nc()
    nc = _NC_CACHE["nc"]

    in_maps = []
    for c in range(N_CORES):
        m = {"x": x[c * B_LOCAL:(c + 1) * B_LOCAL]}
        m.update(weights)
        in_maps.append(m)

    res = run_bass_kernel_spmd(nc, in_maps, core_ids=list(range(N_CORES)), **run_kwargs)
    y = np.concatenate([r["y"] for r in res.results], axis=0)
    kernel.last_result = res
    return y
